# revision 30
# baseline (speedup 1.0000x reference)
"""Trainium2 Bass kernel for nn_CSPNet (GNN message passing over 128 dense graphs).

Strategy: data-parallel over graphs (16 graphs / core on 8 cores). Everything is
computed feature-major ([feature, item] with features on SBUF partitions):

  h0   = latent_w.T @ [emb ; t_rep]                     (PE, bf16, psum-accum)
  demb = sin(2*pi*frac(f*dx + c))                       (DVE sub+mod, ACT Sin)
  e1   = silu(W1d.T@demb + W1a.T@h[src] + W1b.T@h[dst] + (lat_ips@W1c + b1))
         -- all three matmuls accumulate into one PSUM bank; the src/dst
            gathers are stride-0 broadcast access patterns on h (dense graphs);
            the per-graph lattice term + bias ride in as the ACT bias vector.
  e2   = silu(W2.T@e1 + b2)
  agg  = mean_j e2  (DVE bf16 tree reduction over 32-edge groups)
  m    = silu(Wn2.T@silu(Wn1h.T@h + (Wn1a/32).T@agg + bn1) + bn2); h += m
  outs = coord_w.T@h (fp32), (lattice_w/32).T@(group-sum h) (fp32) + host 3x3 bmm

Even graphs live at partitions 0:60, odd at 64:124 for the demb operands so the
sin/mod/sub pipeline runs with ~full partition occupancy.
"""

import sys

if "/opt/trn_rl_repo" not in sys.path:
    sys.path.insert(0, "/opt/trn_rl_repo")

import numpy as np
import ml_dtypes

B = 128          # graphs
N = 32           # nodes per graph
H = 128          # hidden
LATENT = 256
NLAYERS = 4
NFREQ = 10
CORES = 8
G = B // CORES           # graphs per core (16)
NODES = G * N            # nodes per core (512)
EDGES = G * N * N        # edges per core (16384)
GP = G // 2              # graph pairs per core (8)

BF16 = ml_dtypes.bfloat16

TRACE = False            # set True from test harness to collect NTFF exec time
LAST_RESULTS = None      # BassKernelResults of the last run (for profiling)
SIM_SILU_EMULATE = False  # CoreSim lacks Silu; emulate via sigmoid*x for sim runs

_BUILT = None            # cached (nc,) so repeat calls don't re-trace


def _build_nc():
    import concourse.tile as tile
    from concourse import mybir, bacc

    F32 = mybir.dt.float32
    BF = mybir.dt.bfloat16
    AF = mybir.ActivationFunctionType
    ALU = mybir.AluOpType
    X = mybir.AxisListType.X

    nc = bacc.Bacc(None, target_bir_lowering=False, debug=False)

    def din(name, shape, dt=F32):
        return nc.dram_tensor(name, shape, dt, kind="ExternalInput").ap()

    # --- DRAM inputs (per-core slices prepared on host) ---
    embT = din("embT", [H, NODES], BF)            # emb_table gathered, transposed
    trep0 = din("trep0", [H, NODES], BF)          # t[node2graph].T rows 0:128
    trep1 = din("trep1", [H, NODES], BF)          # rows 128:256
    lw_e = din("lw_e", [H, H], BF)                # latent_w rows 0:128
    lw_t0 = din("lw_t0", [H, H], BF)
    lw_t1 = din("lw_t1", [H, H], BF)
    latb = din("latb", [H, 1])
    sfs = din("sfs", [H, GP * N])                 # f*frac (src rows), padded
    sfd = din("sfd", [H, GP * N])                 # f*frac + (s?0.25:0) + 16.5 (dst)
    w1d = din("w1d", [NLAYERS, H, H], BF)         # demb weights (perm'd, dup at 64)
    w1a = din("w1a", [NLAYERS, H, H], BF)
    w1b = din("w1b", [NLAYERS, H, H], BF)
    w2 = din("w2", [NLAYERS, H, H], BF)
    wn1h = din("wn1h", [NLAYERS, H, H], BF)
    wn1a = din("wn1a", [NLAYERS, H, H], BF)       # pre-scaled by 1/32
    wn2 = din("wn2", [NLAYERS, H, H], BF)
    cgb1 = din("cgb1", [H, NLAYERS * G])          # lat_ips@W1c + b1, per (l, g)
    b2c = din("b2c", [H, NLAYERS])
    bn1c = din("bn1c", [H, NLAYERS])
    bn2c = din("bn2c", [H, NLAYERS])
    coordw = din("coordw", [H, 3])
    latws = din("latws", [H, 9])                  # lattice_w / 32

    out_coord = nc.dram_tensor("out_coord", [3, NODES], F32, kind="ExternalOutput").ap()
    out_lat = nc.dram_tensor("out_lat", [9, G], F32, kind="ExternalOutput").ap()

    with tile.TileContext(nc) as tc:
        with (
            tc.tile_pool(name="consts", bufs=1) as cp,
            tc.tile_pool(name="dembp", bufs=2) as dp,
            tc.tile_pool(name="hp", bufs=8) as hp,
            tc.tile_pool(name="work", bufs=3) as wp,
            tc.tile_pool(name="e1pool", bufs=18) as e1p,
            tc.tile_pool(name="e2p", bufs=4) as e2p,
            tc.tile_pool(name="treep", bufs=2) as tp,
            tc.tile_pool(name="psum", bufs=4, space="PSUM") as psA,
        ):
            dma = nc.sync.dma_start

            from concourse.bass import _add_dep_helper
            _prev_mm = [None]

            def mm(*args, **kwargs):
                bi = nc.tensor.matmul(*args, **kwargs)
                if _prev_mm[0] is not None:
                    _add_dep_helper(bi.ins, _prev_mm[0], False,
                                    "force PE program order")
                _prev_mm[0] = bi.ins
                return bi

            def mm_break():
                _prev_mm[0] = None

            def act_silu(out, in_, bias):
                if not SIM_SILU_EMULATE:
                    nc.scalar.activation(out=out, in_=in_, func=AF.Silu, bias=bias)
                    return
                shp = [in_.partition_size(), in_.free_size()]
                xx = wp.tile(shp, F32, tag="silu_x")
                nc.scalar.activation(out=xx[:], in_=in_, func=AF.Identity, bias=bias)
                sg = wp.tile(shp, F32, tag="silu_s")
                nc.scalar.activation(out=sg[:], in_=in_, func=AF.Sigmoid, bias=bias)
                nc.vector.tensor_tensor(out=out, in0=xx[:], in1=sg[:], op=ALU.mult)

            _dma_engines = [nc.sync, nc.gpsimd]
            _dma_i = [0]

            def load(ap, shape, dt=F32, name=None):
                t = cp.tile(shape, dt, tag=name)
                eng = _dma_engines[_dma_i[0] % len(_dma_engines)]
                _dma_i[0] += 1
                eng.dma_start(out=t[:], in_=ap)
                return t

            c_sfs = load(sfs, [H, GP * N], F32, "sfs")
            c_sfd = load(sfd, [H, GP * N], F32, "sfd")
            c_embT = load(embT, [H, NODES], BF, "embT")
            c_lw_e = load(lw_e, [H, H], BF, "lw_e")
            c_lw_t0 = load(lw_t0, [H, H], BF, "lw_t0")
            c_lw_t1 = load(lw_t1, [H, H], BF, "lw_t1")
            c_trep0 = load(trep0, [H, NODES], BF, "trep0")
            c_trep1 = load(trep1, [H, NODES], BF, "trep1")
            c_latb = load(latb, [H, 1], F32, "latb")
            def load_sc(ap, shape, dt, name):
                t = cp.tile(shape, dt, tag=name)
                nc.scalar.dma_start(out=t[:], in_=ap)
                return t

            c_w1a = [load_sc(w1a[0], [H, H], BF, "w1a0")]
            c_w1b = [load_sc(w1b[0], [H, H], BF, "w1b0")]
            c_w1d = [load_sc(w1d[0], [H, H], BF, "w1d0")]
            c_w2 = [load_sc(w2[0], [H, H], BF, "w20")]
            c_cgb1 = load(cgb1, [H, NLAYERS * G], F32, "cgb1")
            c_b2c = load(b2c, [H, NLAYERS], F32, "b2c")
            for l in range(1, NLAYERS):
                c_w1a.append(load(w1a[l], [H, H], BF, f"w1a{l}"))
                c_w1b.append(load(w1b[l], [H, H], BF, f"w1b{l}"))
                c_w1d.append(load(w1d[l], [H, H], BF, f"w1d{l}"))
                c_w2.append(load(w2[l], [H, H], BF, f"w2{l}"))
            c_wn1h = [load(wn1h[l], [H, H], BF, f"wn1h{l}") for l in range(NLAYERS)]
            c_wn1a = [load(wn1a[l], [H, H], BF, f"wn1a{l}") for l in range(NLAYERS)]
            c_wn2 = [load(wn2[l], [H, H], BF, f"wn2{l}") for l in range(NLAYERS)]
            c_bn1c = load(bn1c, [H, NLAYERS], F32, "bn1c")
            c_bn2c = load(bn2c, [H, NLAYERS], F32, "bn2c")
            c_coordw = load(coordw, [H, 3], F32, "coordw")
            c_latws = load(latws, [H, 9], F32, "latws")

            # --- demb chunks, one per graph pair [128, 1024] ---
            db2s = []
            for p in range(GP):
                fdq = dp.tile([H, 1024], F32, tag="fd")
                nc.vector.tensor_tensor(
                    out=fdq[:].rearrange("p (i j) -> p i j", i=N),
                    in0=c_sfd[:, 32 * p:32 * (p + 1)].unsqueeze(1).broadcast_to([H, N, N]),
                    in1=c_sfs[:, 32 * p:32 * (p + 1)].unsqueeze(2).broadcast_to([H, N, N]),
                    op=ALU.subtract,
                )
                kk = dp.tile([H, 1024], mybir.dt.int32, tag="kk")
                nc.vector.tensor_copy(out=kk[:], in_=fdq[:])
                nc.vector.tensor_tensor(out=fdq[:], in0=fdq[:], in1=kk[:], op=ALU.subtract)
                db2q = cp.tile([H, 1024], BF, tag=f"db2_{p}", name=f"db2_{p}")
                nc.scalar.activation(out=db2q[:], in_=fdq[:], func=AF.Sin,
                                     scale=float(2 * np.pi))
                db2s.append(db2q)

            # --- h0 per quad ---
            hq_f32 = []
            hq_bf = []
            for q in range(4):
                csl = slice(128 * q, 128 * (q + 1))
                ph = psA.tile([H, 1024], F32, tag="psA", name=f"ph0_{q}")
                mm(ph[:, 0:128], lhsT=c_lw_e[:], rhs=c_embT[:, csl], start=True, stop=False)
                mm(ph[:, 0:128], lhsT=c_lw_t0[:], rhs=c_trep0[:, csl], start=False, stop=False)
                mm(ph[:, 0:128], lhsT=c_lw_t1[:], rhs=c_trep1[:, csl], start=False, stop=True)
                hf = cp.tile([H, 128], F32, tag=f"h_f32_{q}")
                nc.scalar.activation(out=hf[:], in_=ph[:, 0:128], func=AF.Identity, bias=c_latb[:])
                hb = hp.tile([H, 128], BF, tag="h_bf", name=f"h_bf_{q}_init")
                nc.gpsimd.tensor_copy(out=hb[:], in_=hf[:])
                hq_f32.append(hf)
                hq_bf.append(hb)

            for l in range(NLAYERS):
                # phase 1+2 interleaved by pair: e1(p), then e2(p-1)
                e1s = [None] * G
                e2qs = [None] * 4

                def emit_e1(pi):
                    gpair = (2 * pi, 2 * pi + 1)
                    pe1s = {}
                    for gx in gpair:
                        pe1s[gx] = psA.tile([H, 1024], F32, tag="psA", name=f"pe1_{l}_{gx}")
                    for gx in gpair:
                        q, gg = gx // 4, gx % 4
                        hb = hq_bf[q]
                        for hh in range(2):
                            mm(
                                pe1s[gx][:, 512 * hh:512 * (hh + 1)], lhsT=c_w1a[l][:],
                                rhs=hb[:, 32 * gg + 16 * hh: 32 * gg + 16 * hh + 16]
                                    .unsqueeze(2).broadcast_to([H, 16, N]),
                                start=True, stop=False)
                    for gx in gpair:
                        q, gg = gx // 4, gx % 4
                        hb = hq_bf[q]
                        for hh in range(2):
                            mm(
                                pe1s[gx][:, 512 * hh:512 * (hh + 1)], lhsT=c_w1b[l][:],
                                rhs=hb[:, 32 * gg: 32 * gg + 32]
                                    .unsqueeze(1).broadcast_to([H, 16, N]),
                                start=False, stop=False)
                    for gx in gpair:
                        q, gg = gx // 4, gx % 4
                        base = 64 * (gg % 2)
                        for hh in range(2):
                            mm(
                                pe1s[gx][:, 512 * hh:512 * (hh + 1)],
                                lhsT=c_w1d[l][base:base + 60, :],
                                rhs=db2s[gx // 2][base:base + 60, 512 * hh: 512 * (hh + 1)],
                                start=False, stop=True)
                    for gx in gpair:
                        e1 = e1p.tile([H, 1024], BF, tag="e1", name=f"e1_{l}_{gx}")
                        act_silu(e1[:], pe1s[gx][:], c_cgb1[:, G * l + gx: G * l + gx + 1])
                        e1s[gx] = e1

                def emit_e2(pi):
                    for gx in (2 * pi, 2 * pi + 1):
                        q, gg = gx // 4, gx % 4
                        if gg == 0:
                            e2qs[q] = e2p.tile([H, 4096], BF, tag="e2q", name=f"e2q_{l}_{q}")
                        pe2 = psA.tile([H, 1024], F32, tag="psA", name=f"pe2_{l}_{gx}")
                        for hh in range(2):
                            mm(
                                pe2[:, 512 * hh:512 * (hh + 1)], lhsT=c_w2[l][:],
                                rhs=e1s[gx][:, 512 * hh:512 * (hh + 1)], start=True, stop=True)
                        act_silu(e2qs[q][:, 1024 * gg:1024 * (gg + 1)], pe2[:], c_b2c[:, l:l + 1])

                emit_e1(0)
                for pi in range(1, G // 2):
                    emit_e1(pi)
                    emit_e2(pi - 1)
                emit_e2(G // 2 - 1)
                # phase 3: trees + node MLP per quad
                for q in range(4):
                    e2q = e2qs[q]
                    agg_q = hp.tile([H, 128], BF, tag="agg", name=f"agg_{l}_{q}")
                    t1 = tp.tile([H, 2048], BF, tag="t1")
                    v = e2q[:].rearrange("p (a j) -> p a j", a=128)
                    nc.vector.tensor_tensor(out=t1[:].rearrange("p (a j) -> p a j", a=128),
                                            in0=v[:, :, 0:16], in1=v[:, :, 16:32], op=ALU.add)
                    t2 = tp.tile([H, 1024], BF, tag="t2")
                    v = t1[:].rearrange("p (a j) -> p a j", a=128)
                    nc.vector.tensor_tensor(out=t2[:].rearrange("p (a j) -> p a j", a=128),
                                            in0=v[:, :, 0:8], in1=v[:, :, 8:16], op=ALU.add)
                    t3 = tp.tile([H, 512], BF, tag="t3")
                    v = t2[:].rearrange("p (a j) -> p a j", a=128)
                    nc.vector.tensor_tensor(out=t3[:].rearrange("p (a j) -> p a j", a=128),
                                            in0=v[:, :, 0:4], in1=v[:, :, 4:8], op=ALU.add)
                    t4 = tp.tile([H, 256], BF, tag="t4")
                    v = t3[:].rearrange("p (a j) -> p a j", a=128)
                    nc.vector.tensor_tensor(out=t4[:].rearrange("p (a j) -> p a j", a=128),
                                            in0=v[:, :, 0:2], in1=v[:, :, 2:4], op=ALU.add)
                    v = t4[:].rearrange("p (a j) -> p a j", a=128)
                    nc.vector.tensor_tensor(out=agg_q[:].rearrange("p (a j) -> p a j", a=128),
                                            in0=v[:, :, 0:1], in1=v[:, :, 1:2], op=ALU.add)
                    pn1 = psA.tile([H, 1024], F32, tag="psA", name=f"pn1_{l}_{q}")
                    nc.tensor.matmul(pn1[:, 0:128], lhsT=c_wn1h[l][:], rhs=hq_bf[q][:], start=True, stop=False)
                    nc.tensor.matmul(pn1[:, 0:128], lhsT=c_wn1a[l][:], rhs=agg_q[:], start=False, stop=True)
                    m1 = wp.tile([H, 128], BF, tag="m1")
                    act_silu(m1[:], pn1[:, 0:128], c_bn1c[:, l:l + 1])
                    pn2 = psA.tile([H, 1024], F32, tag="psA", name=f"pn2_{l}_{q}")
                    nc.tensor.matmul(pn2[:, 0:128], lhsT=c_wn2[l][:], rhs=m1[:], start=True, stop=True)
                    m2 = wp.tile([H, 128], F32, tag="m2")
                    act_silu(m2[:], pn2[:, 0:128], c_bn2c[:, l:l + 1])
                    nc.vector.tensor_tensor(out=hq_f32[q][:], in0=hq_f32[q][:], in1=m2[:], op=ALU.add)
                    hb = hp.tile([H, 128], BF, tag="h_bf", name=f"h_bf_{q}_{l}")
                    nc.gpsimd.tensor_copy(out=hb[:], in_=hq_f32[q][:])
                    hq_bf[q] = hb

            # --- outputs ---
            h_all = cp.tile([H, NODES], F32, tag="h_all")
            for q in range(4):
                nc.gpsimd.tensor_copy(out=h_all[:, 128 * q:128 * (q + 1)], in_=hq_f32[q][:])
            pc = psA.tile([H, 1024], F32, tag="psA")
            mm(pc[0:3, 0:512], lhsT=c_coordw[:], rhs=h_all[:], start=True, stop=True)
            sb_c = wp.tile([3, NODES], F32, tag="sbc")
            nc.scalar.copy(sb_c[:], pc[0:3, 0:512])
            dma(out=out_coord, in_=sb_c[:])

            gf = wp.tile([H, G], F32, tag="gf")
            nc.vector.tensor_reduce(out=gf[:], in_=h_all[:].rearrange("p (g n) -> p g n", g=G),
                                    axis=X, op=ALU.add)
            pl_ = psA.tile([H, 1024], F32, tag="psA")
            mm(pl_[0:9, 0:G], lhsT=c_latws[:], rhs=gf[:], start=True, stop=True)
            sb_l = wp.tile([9, G], F32, tag="sbl")
            nc.vector.tensor_copy(out=sb_l[:], in_=pl_[0:9, 0:G])
            dma(out=out_lat, in_=sb_l[:])

    nc.finalize()
    return nc


def _host_prep(inputs):
    """Validate graph structure and build the per-core input maps."""
    t = np.asarray(inputs["t"], np.float32)
    frac = np.asarray(inputs["frac_coords"], np.float32)
    lattices = np.asarray(inputs["lattices"], np.float32)
    emb_table = np.asarray(inputs["emb_table"], np.float32)
    latent_w = np.asarray(inputs["latent_w"], np.float32)
    latent_b = np.asarray(inputs["latent_b"], np.float32)
    edge_w1 = np.asarray(inputs["edge_w1"], np.float32)
    edge_b1 = np.asarray(inputs["edge_b1"], np.float32)
    edge_w2 = np.asarray(inputs["edge_w2"], np.float32)
    edge_b2 = np.asarray(inputs["edge_b2"], np.float32)
    node_w1 = np.asarray(inputs["node_w1"], np.float32)
    node_b1 = np.asarray(inputs["node_b1"], np.float32)
    node_w2 = np.asarray(inputs["node_w2"], np.float32)
    node_b2 = np.asarray(inputs["node_b2"], np.float32)
    coord_w = np.asarray(inputs["coord_w"], np.float32)
    lattice_w = np.asarray(inputs["lattice_w"], np.float32)
    atom_types = np.asarray(inputs["atom_types"], np.int32)
    node2graph = np.asarray(inputs["node2graph"], np.int32)
    edge_index = np.asarray(inputs["edge_index"], np.int32)
    edge2graph = np.asarray(inputs["edge2graph"], np.int32)

    # structured-input check (dense per-graph edges, row-major)
    base = np.arange(N)
    ii = np.repeat(base, N)
    jj = np.tile(base, N)
    off = (np.arange(B) * N)[:, None]
    src_exp = (ii[None, :] + off).reshape(-1)
    dst_exp = (jj[None, :] + off).reshape(-1)
    structured = (
        np.array_equal(edge_index[0], src_exp)
        and np.array_equal(edge_index[1], dst_exp)
        and np.array_equal(node2graph, np.repeat(np.arange(B), N))
        and np.array_equal(edge2graph, np.repeat(np.arange(B), N * N))
    )
    if not structured:
        return None

    # gathers (host): embeddings + latent conditioning rows
    emb_g = emb_table[atom_types - 1]                      # [4096, H]
    t_rep = t[node2graph]                                  # [4096, LATENT]
    lat_ips = np.einsum("bij,bkj->bik", lattices, lattices).reshape(B, 9)

    # demb weight row permutation: device row r = 30s + 3f + c  <-  ref 30s+10c+f
    perm = np.empty(60, np.int64)
    for s in range(2):
        for f in range(NFREQ):
            for c in range(3):
                perm[30 * s + 3 * f + c] = 30 * s + 10 * c + f

    def dup64(a60):   # [60, H] -> [128, H] duplicated at partition 64
        out = np.zeros((H, H), np.float32)
        out[0:60] = a60
        out[64:124] = a60
        return out

    w1d_np = np.stack([dup64(edge_w1[l][265:325][perm]) for l in range(NLAYERS)])
    w1a_np = np.stack([edge_w1[l][0:H] for l in range(NLAYERS)])
    w1b_np = np.stack([edge_w1[l][H:2 * H] for l in range(NLAYERS)])
    w1c = np.stack([edge_w1[l][2 * H:2 * H + 9] for l in range(NLAYERS)])  # [L, 9, H]
    wn1h_np = np.stack([node_w1[l][0:H] for l in range(NLAYERS)])
    wn1a_np = np.stack([node_w1[l][H:2 * H] / np.float32(N) for l in range(NLAYERS)])

    freqs = np.arange(NFREQ, dtype=np.float64)             # integer freqs (2*pi folded out)

    in_maps = []
    for core in range(CORES):
        g0 = core * G
        nsl = slice(g0 * N, (g0 + G) * N)
        frac_c = frac[nsl]                                 # [512, 3]
        # sfs/sfd [128, GP*N]: col = 32*(pair) + node; partition<64 -> even graph
        sfs_np = np.zeros((H, GP * N), np.float32)
        sfd_np = np.zeros((H, GP * N), np.float32)
        for par in range(2):
            rbase = 64 * par
            for s in range(2):
                for f in range(NFREQ):
                    for c in range(3):
                        r = rbase + 30 * s + 3 * f + c
                        for p in range(GP):
                            nn0 = (2 * p + par) * N
                            x = frac_c[nn0:nn0 + N, c].astype(np.float64)
                            sfs_np[r, p * N:(p + 1) * N] = (freqs[f] * x).astype(np.float32)
                            sfd_np[r, p * N:(p + 1) * N] = (freqs[f] * x + (0.25 if s else 0.0)).astype(np.float32)

        cgb1_np = np.zeros((H, NLAYERS * G), np.float32)
        for l in range(NLAYERS):
            cg = lat_ips[g0:g0 + G] @ w1c[l] + edge_b1[l]  # [G, H]
            cgb1_np[:, G * l:G * (l + 1)] = cg.T

        m = {
            "embT": np.ascontiguousarray(emb_g[nsl].T).astype(BF16),
            "trep0": np.ascontiguousarray(t_rep[nsl, 0:H].T).astype(BF16),
            "trep1": np.ascontiguousarray(t_rep[nsl, H:LATENT].T).astype(BF16),
            "lw_e": latent_w[0:H].astype(BF16),
            "lw_t0": latent_w[H:2 * H].astype(BF16),
            "lw_t1": latent_w[2 * H:3 * H].astype(BF16),
            "latb": latent_b.reshape(H, 1),
            "sfs": sfs_np,
            "sfd": sfd_np,
            "w1d": w1d_np.astype(BF16),
            "w1a": w1a_np.astype(BF16),
            "w1b": w1b_np.astype(BF16),
            "w2": edge_w2.astype(BF16),
            "wn1h": wn1h_np.astype(BF16),
            "wn1a": wn1a_np.astype(BF16),
            "wn2": node_w2.astype(BF16),
            "cgb1": cgb1_np,
            "b2c": np.ascontiguousarray(edge_b2.T),
            "bn1c": np.ascontiguousarray(node_b1.T),
            "bn2c": np.ascontiguousarray(node_b2.T),
            "coordw": coord_w,
            "latws": lattice_w / np.float32(N),
        }
        in_maps.append(m)
    return in_maps, lattices


def _reference_fallback(inputs):
    """Pure-numpy fallback for non-structured inputs (not expected in practice)."""
    t = np.asarray(inputs["t"], np.float32)
    frac = np.asarray(inputs["frac_coords"], np.float32)
    lattices = np.asarray(inputs["lattices"], np.float32)
    silu = lambda x: x / (1.0 + np.exp(-x))
    emb = np.asarray(inputs["emb_table"], np.float32)
    atom_types = np.asarray(inputs["atom_types"], np.int32)
    node2graph = np.asarray(inputs["node2graph"], np.int32)
    edge_index = np.asarray(inputs["edge_index"], np.int32)
    edge2graph = np.asarray(inputs["edge2graph"], np.int32)
    src, dst = edge_index
    nnodes = frac.shape[0]
    h = emb[atom_types - 1]
    h = np.concatenate([h, t[node2graph]], 1) @ np.asarray(inputs["latent_w"]) + np.asarray(inputs["latent_b"])
    fd = np.mod(frac[dst] - frac[src], 1.0)
    freqs = 2 * np.pi * np.arange(NFREQ, dtype=np.float32)
    demb = (fd[..., None] * freqs).reshape(-1, NFREQ * 3)
    demb = np.concatenate([np.sin(demb), np.cos(demb)], -1)
    lat_ips = np.einsum("bij,bkj->bik", lattices, lattices).reshape(-1, 9)
    lat_e = lat_ips[edge2graph]
    deg = np.maximum(np.bincount(src, minlength=nnodes).astype(np.float32), 1.0)
    for l in range(NLAYERS):
        e = np.concatenate([h[src], h[dst], lat_e, demb], 1)
        e = silu(e @ np.asarray(inputs["edge_w1"])[l] + np.asarray(inputs["edge_b1"])[l])
        e = silu(e @ np.asarray(inputs["edge_w2"])[l] + np.asarray(inputs["edge_b2"])[l])
        agg = np.zeros((nnodes, H), np.float32)
        np.add.at(agg, src, e)
        agg = agg / deg[:, None]
        m = np.concatenate([h, agg], 1)
        m = silu(m @ np.asarray(inputs["node_w1"])[l] + np.asarray(inputs["node_b1"])[l])
        m = silu(m @ np.asarray(inputs["node_w2"])[l] + np.asarray(inputs["node_b2"])[l])
        h = h + m
    coord_out = h @ np.asarray(inputs["coord_w"])
    gcount = np.maximum(np.bincount(node2graph, minlength=B).astype(np.float32), 1.0)
    gfeat = np.zeros((B, H), np.float32)
    np.add.at(gfeat, node2graph, h)
    gfeat = gfeat / gcount[:, None]
    lattice_out = (gfeat @ np.asarray(inputs["lattice_w"])).reshape(-1, 3, 3)
    lattice_out = np.einsum("bij,bjk->bik", lattice_out, lattices)
    return lattice_out.astype(np.float32), coord_out.astype(np.float32)


def kernel(**inputs):
    global _BUILT, LAST_RESULTS
    prep = _host_prep(inputs)
    if prep is None:
        return _reference_fallback(inputs)
    in_maps, lattices = prep

    from concourse.bass_utils import run_bass_kernel_spmd

    if _BUILT is None:
        _BUILT = _build_nc()
    nc = _BUILT

    res = run_bass_kernel_spmd(nc, in_maps, core_ids=list(range(CORES)), trace=TRACE)
    LAST_RESULTS = res

    coord_out = np.empty((B * N, 3), np.float32)
    lattice_out = np.empty((B, 3, 3), np.float32)
    for core in range(CORES):
        r = res.results[core]
        coord_out[core * NODES:(core + 1) * NODES] = r["out_coord"].T
        pre = np.ascontiguousarray(r["out_lat"].T).reshape(G, 3, 3)
        gsl = slice(core * G, (core + 1) * G)
        lattice_out[gsl] = np.einsum("bij,bjk->bik", pre, lattices[gsl])
    return lattice_out, coord_out


# revision 33
# speedup vs baseline: 1.0023x; 1.0023x over previous
"""Trainium2 Bass kernel for nn_CSPNet (GNN message passing over 128 dense graphs).

Strategy: data-parallel over graphs (16 graphs / core on 8 cores). Everything is
computed feature-major ([feature, item] with features on SBUF partitions):

  h0   = latent_w.T @ [emb ; t_rep]                     (PE, bf16, psum-accum)
  demb = sin(2*pi*frac(f*dx + c))                       (DVE sub+mod, ACT Sin)
  e1   = silu(W1d.T@demb + W1a.T@h[src] + W1b.T@h[dst] + (lat_ips@W1c + b1))
         -- all three matmuls accumulate into one PSUM bank; the src/dst
            gathers are stride-0 broadcast access patterns on h (dense graphs);
            the per-graph lattice term + bias ride in as the ACT bias vector.
  e2   = silu(W2.T@e1 + b2)
  agg  = mean_j e2  (DVE bf16 tree reduction over 32-edge groups)
  m    = silu(Wn2.T@silu(Wn1h.T@h + (Wn1a/32).T@agg + bn1) + bn2); h += m
  outs = coord_w.T@h (fp32), (lattice_w/32).T@(group-sum h) (fp32) + host 3x3 bmm

Even graphs live at partitions 0:60, odd at 64:124 for the demb operands so the
sin/mod/sub pipeline runs with ~full partition occupancy.
"""

import sys

if "/opt/trn_rl_repo" not in sys.path:
    sys.path.insert(0, "/opt/trn_rl_repo")

import numpy as np
import ml_dtypes

B = 128          # graphs
N = 32           # nodes per graph
H = 128          # hidden
LATENT = 256
NLAYERS = 4
NFREQ = 10
CORES = 8
G = B // CORES           # graphs per core (16)
NODES = G * N            # nodes per core (512)
EDGES = G * N * N        # edges per core (16384)
GP = G // 2              # graph pairs per core (8)

BF16 = ml_dtypes.bfloat16

TRACE = False            # set True from test harness to collect NTFF exec time
LAST_RESULTS = None      # BassKernelResults of the last run (for profiling)
SIM_SILU_EMULATE = False  # CoreSim lacks Silu; emulate via sigmoid*x for sim runs

_BUILT = None            # cached (nc,) so repeat calls don't re-trace


def _build_nc():
    import concourse.tile as tile
    from concourse import mybir, bacc

    F32 = mybir.dt.float32
    BF = mybir.dt.bfloat16
    AF = mybir.ActivationFunctionType
    ALU = mybir.AluOpType
    X = mybir.AxisListType.X

    nc = bacc.Bacc(None, target_bir_lowering=False, debug=False)

    def din(name, shape, dt=F32):
        return nc.dram_tensor(name, shape, dt, kind="ExternalInput").ap()

    # --- DRAM inputs (per-core slices prepared on host) ---
    embT = din("embT", [H, NODES], BF)            # emb_table gathered, transposed
    trep0 = din("trep0", [H, NODES], BF)          # t[node2graph].T rows 0:128
    trep1 = din("trep1", [H, NODES], BF)          # rows 128:256
    lw_e = din("lw_e", [H, H], BF)                # latent_w rows 0:128
    lw_t0 = din("lw_t0", [H, H], BF)
    lw_t1 = din("lw_t1", [H, H], BF)
    latb = din("latb", [H, 1])
    sfs = din("sfs", [H, GP * N])                 # f*frac (src rows), padded
    sfd = din("sfd", [H, GP * N])                 # f*frac + (s?0.25:0) + 16.5 (dst)
    w1d = din("w1d", [NLAYERS, H, H], BF)         # demb weights (perm'd, dup at 64)
    w1a = din("w1a", [NLAYERS, H, H], BF)
    w1b = din("w1b", [NLAYERS, H, H], BF)
    w2 = din("w2", [NLAYERS, H, H], BF)
    wn1h = din("wn1h", [NLAYERS, H, H], BF)
    wn1a = din("wn1a", [NLAYERS, H, H], BF)       # pre-scaled by 1/32
    wn2 = din("wn2", [NLAYERS, H, H], BF)
    cgb1 = din("cgb1", [H, NLAYERS * G])          # lat_ips@W1c + b1, per (l, g)
    b2c = din("b2c", [H, NLAYERS])
    bn1c = din("bn1c", [H, NLAYERS])
    bn2c = din("bn2c", [H, NLAYERS])
    coordw = din("coordw", [H, 3])
    latws = din("latws", [H, 9])                  # lattice_w / 32

    out_coord = nc.dram_tensor("out_coord", [3, NODES], F32, kind="ExternalOutput").ap()
    out_lat = nc.dram_tensor("out_lat", [9, G], F32, kind="ExternalOutput").ap()

    with tile.TileContext(nc) as tc:
        with (
            tc.tile_pool(name="consts", bufs=1) as cp,
            tc.tile_pool(name="dembp", bufs=2) as dp,
            tc.tile_pool(name="hp", bufs=8) as hp,
            tc.tile_pool(name="work", bufs=3) as wp,
            tc.tile_pool(name="e1pool", bufs=18) as e1p,
            tc.tile_pool(name="e2p", bufs=4) as e2p,
            tc.tile_pool(name="treep", bufs=2) as tp,
            tc.tile_pool(name="psum", bufs=4, space="PSUM") as psA,
        ):
            dma = nc.sync.dma_start

            from concourse.bass import _add_dep_helper
            _prev_mm = [None]

            def mm(*args, **kwargs):
                bi = nc.tensor.matmul(*args, **kwargs)
                if _prev_mm[0] is not None:
                    _add_dep_helper(bi.ins, _prev_mm[0], False,
                                    "force PE program order")
                _prev_mm[0] = bi.ins
                return bi

            def mm_break():
                _prev_mm[0] = None

            def act_silu(out, in_, bias):
                if not SIM_SILU_EMULATE:
                    nc.scalar.activation(out=out, in_=in_, func=AF.Silu, bias=bias)
                    return
                shp = [in_.partition_size(), in_.free_size()]
                xx = wp.tile(shp, F32, tag="silu_x")
                nc.scalar.activation(out=xx[:], in_=in_, func=AF.Identity, bias=bias)
                sg = wp.tile(shp, F32, tag="silu_s")
                nc.scalar.activation(out=sg[:], in_=in_, func=AF.Sigmoid, bias=bias)
                nc.vector.tensor_tensor(out=out, in0=xx[:], in1=sg[:], op=ALU.mult)

            _dma_engines = [nc.sync, nc.gpsimd]
            _dma_i = [0]

            def load(ap, shape, dt=F32, name=None):
                t = cp.tile(shape, dt, tag=name)
                eng = _dma_engines[_dma_i[0] % len(_dma_engines)]
                _dma_i[0] += 1
                eng.dma_start(out=t[:], in_=ap)
                return t

            c_sfs = load(sfs, [H, GP * N], F32, "sfs")
            c_sfd = load(sfd, [H, GP * N], F32, "sfd")
            c_embT = load(embT, [H, NODES], BF, "embT")
            c_lw_e = load(lw_e, [H, H], BF, "lw_e")
            c_lw_t0 = load(lw_t0, [H, H], BF, "lw_t0")
            c_lw_t1 = load(lw_t1, [H, H], BF, "lw_t1")
            c_trep0 = load(trep0, [H, NODES], BF, "trep0")
            c_trep1 = load(trep1, [H, NODES], BF, "trep1")
            c_latb = load(latb, [H, 1], F32, "latb")
            def load_sc(ap, shape, dt, name):
                t = cp.tile(shape, dt, tag=name)
                nc.scalar.dma_start(out=t[:], in_=ap)
                return t

            c_w1a = [load_sc(w1a[0], [H, H], BF, "w1a0")]
            c_w1b = [load_sc(w1b[0], [H, H], BF, "w1b0")]
            c_w1d = [load_sc(w1d[0], [H, H], BF, "w1d0")]
            c_w2 = [load_sc(w2[0], [H, H], BF, "w20")]
            c_cgb1 = load(cgb1, [H, NLAYERS * G], F32, "cgb1")
            c_b2c = load(b2c, [H, NLAYERS], F32, "b2c")
            for l in range(1, NLAYERS):
                c_w1a.append(load(w1a[l], [H, H], BF, f"w1a{l}"))
                c_w1b.append(load(w1b[l], [H, H], BF, f"w1b{l}"))
                c_w1d.append(load(w1d[l], [H, H], BF, f"w1d{l}"))
                c_w2.append(load(w2[l], [H, H], BF, f"w2{l}"))
            c_wn1h = [load(wn1h[l], [H, H], BF, f"wn1h{l}") for l in range(NLAYERS)]
            c_wn1a = [load(wn1a[l], [H, H], BF, f"wn1a{l}") for l in range(NLAYERS)]
            c_wn2 = [load(wn2[l], [H, H], BF, f"wn2{l}") for l in range(NLAYERS)]
            c_bn1c = load(bn1c, [H, NLAYERS], F32, "bn1c")
            c_bn2c = load(bn2c, [H, NLAYERS], F32, "bn2c")
            c_coordw = load(coordw, [H, 3], F32, "coordw")
            c_latws = load(latws, [H, 9], F32, "latws")

            # --- demb chunks, one per graph pair [128, 1024] ---
            db2s = []
            for p in range(GP):
                fdq = dp.tile([H, 1024], F32, tag="fd")
                nc.vector.tensor_tensor(
                    out=fdq[:].rearrange("p (i j) -> p i j", i=N),
                    in0=c_sfd[:, 32 * p:32 * (p + 1)].unsqueeze(1).broadcast_to([H, N, N]),
                    in1=c_sfs[:, 32 * p:32 * (p + 1)].unsqueeze(2).broadcast_to([H, N, N]),
                    op=ALU.subtract,
                )
                kk = dp.tile([H, 1024], mybir.dt.int32, tag="kk")
                nc.vector.tensor_copy(out=kk[:], in_=fdq[:])
                nc.vector.tensor_tensor(out=fdq[:], in0=fdq[:], in1=kk[:], op=ALU.subtract)
                db2q = cp.tile([H, 1024], BF, tag=f"db2_{p}", name=f"db2_{p}")
                nc.scalar.activation(out=db2q[:], in_=fdq[:], func=AF.Sin,
                                     scale=float(2 * np.pi))
                db2s.append(db2q)

            # --- h0 per quad ---
            hq_f32 = []
            hq_bf = []
            for q in range(4):
                csl = slice(128 * q, 128 * (q + 1))
                ph = psA.tile([H, 1024], F32, tag="psA", name=f"ph0_{q}")
                mm(ph[:, 0:128], lhsT=c_lw_e[:], rhs=c_embT[:, csl], start=True, stop=False)
                mm(ph[:, 0:128], lhsT=c_lw_t0[:], rhs=c_trep0[:, csl], start=False, stop=False)
                mm(ph[:, 0:128], lhsT=c_lw_t1[:], rhs=c_trep1[:, csl], start=False, stop=True)
                hf = cp.tile([H, 128], F32, tag=f"h_f32_{q}")
                nc.scalar.activation(out=hf[:], in_=ph[:, 0:128], func=AF.Identity, bias=c_latb[:])
                hb = hp.tile([H, 128], BF, tag="h_bf", name=f"h_bf_{q}_init")
                nc.gpsimd.tensor_copy(out=hb[:], in_=hf[:])
                hq_f32.append(hf)
                hq_bf.append(hb)

            for l in range(NLAYERS):
                # phase 1+2 interleaved by pair: e1(p), then e2(p-1)
                e1s = [None] * G
                e2qs = [None] * 4

                def emit_e1(pi):
                    gpair = (2 * pi, 2 * pi + 1)
                    pe1s = {}
                    for gx in gpair:
                        pe1s[gx] = psA.tile([H, 1024], F32, tag="psA", name=f"pe1_{l}_{gx}")
                    for gx in gpair:
                        q, gg = gx // 4, gx % 4
                        hb = hq_bf[q]
                        for hh in range(2):
                            mm(
                                pe1s[gx][:, 512 * hh:512 * (hh + 1)], lhsT=c_w1a[l][:],
                                rhs=hb[:, 32 * gg + 16 * hh: 32 * gg + 16 * hh + 16]
                                    .unsqueeze(2).broadcast_to([H, 16, N]),
                                start=True, stop=False)
                    for gx in gpair:
                        q, gg = gx // 4, gx % 4
                        hb = hq_bf[q]
                        for hh in range(2):
                            mm(
                                pe1s[gx][:, 512 * hh:512 * (hh + 1)], lhsT=c_w1b[l][:],
                                rhs=hb[:, 32 * gg: 32 * gg + 32]
                                    .unsqueeze(1).broadcast_to([H, 16, N]),
                                start=False, stop=False)
                    for gx in gpair:
                        q, gg = gx // 4, gx % 4
                        base = 64 * (gg % 2)
                        for hh in range(2):
                            mm(
                                pe1s[gx][:, 512 * hh:512 * (hh + 1)],
                                lhsT=c_w1d[l][base:base + 60, :],
                                rhs=db2s[gx // 2][base:base + 60, 512 * hh: 512 * (hh + 1)],
                                start=False, stop=True)
                    for gx in gpair:
                        e1 = e1p.tile([H, 1024], BF, tag="e1", name=f"e1_{l}_{gx}")
                        act_silu(e1[:], pe1s[gx][:], c_cgb1[:, G * l + gx: G * l + gx + 1])
                        e1s[gx] = e1

                def emit_e2(pi):
                    for gx in (2 * pi, 2 * pi + 1):
                        q, gg = gx // 4, gx % 4
                        if gg == 0:
                            e2qs[q] = e2p.tile([H, 4096], BF, tag="e2q", name=f"e2q_{l}_{q}")
                        pe2 = psA.tile([H, 1024], F32, tag="psA", name=f"pe2_{l}_{gx}")
                        for hh in range(2):
                            mm(
                                pe2[:, 512 * hh:512 * (hh + 1)], lhsT=c_w2[l][:],
                                rhs=e1s[gx][:, 512 * hh:512 * (hh + 1)], start=True, stop=True)
                        act_silu(e2qs[q][:, 1024 * gg:1024 * (gg + 1)], pe2[:], c_b2c[:, l:l + 1])

                emit_e1(0)
                for pi in range(1, G // 2):
                    emit_e1(pi)
                    emit_e2(pi - 1)
                emit_e2(G // 2 - 1)
                # phase 3: trees + node MLP per quad
                for q in range(4):
                    e2q = e2qs[q]
                    agg_q = hp.tile([H, 128], BF, tag="agg", name=f"agg_{l}_{q}")
                    t1 = tp.tile([H, 2048], BF, tag="t1")
                    v = e2q[:].rearrange("p (a j) -> p a j", a=128)
                    nc.vector.tensor_tensor(out=t1[:].rearrange("p (a j) -> p a j", a=128),
                                            in0=v[:, :, 0:16], in1=v[:, :, 16:32], op=ALU.add)
                    t2 = tp.tile([H, 1024], BF, tag="t2")
                    v = t1[:].rearrange("p (a j) -> p a j", a=128)
                    nc.vector.tensor_tensor(out=t2[:].rearrange("p (a j) -> p a j", a=128),
                                            in0=v[:, :, 0:8], in1=v[:, :, 8:16], op=ALU.add)
                    t3 = tp.tile([H, 512], BF, tag="t3")
                    v = t2[:].rearrange("p (a j) -> p a j", a=128)
                    nc.vector.tensor_tensor(out=t3[:].rearrange("p (a j) -> p a j", a=128),
                                            in0=v[:, :, 0:4], in1=v[:, :, 4:8], op=ALU.add)
                    t4 = tp.tile([H, 256], BF, tag="t4")
                    v = t3[:].rearrange("p (a j) -> p a j", a=128)
                    nc.vector.tensor_tensor(out=t4[:].rearrange("p (a j) -> p a j", a=128),
                                            in0=v[:, :, 0:2], in1=v[:, :, 2:4], op=ALU.add)
                    v = t4[:].rearrange("p (a j) -> p a j", a=128)
                    nc.vector.tensor_tensor(out=agg_q[:].rearrange("p (a j) -> p a j", a=128),
                                            in0=v[:, :, 0:1], in1=v[:, :, 1:2], op=ALU.add)
                    pn1 = psA.tile([H, 1024], F32, tag="psA", name=f"pn1_{l}_{q}")
                    nc.tensor.matmul(pn1[:, 0:128], lhsT=c_wn1h[l][:], rhs=hq_bf[q][:], start=True, stop=False)
                    nc.tensor.matmul(pn1[:, 0:128], lhsT=c_wn1a[l][:], rhs=agg_q[:], start=False, stop=True)
                    m1 = wp.tile([H, 128], BF, tag="m1")
                    act_silu(m1[:], pn1[:, 0:128], c_bn1c[:, l:l + 1])
                    pn2 = psA.tile([H, 1024], F32, tag="psA", name=f"pn2_{l}_{q}")
                    nc.tensor.matmul(pn2[:, 0:128], lhsT=c_wn2[l][:], rhs=m1[:], start=True, stop=True)
                    m2 = wp.tile([H, 128], F32, tag="m2")
                    act_silu(m2[:], pn2[:, 0:128], c_bn2c[:, l:l + 1])
                    nc.vector.tensor_tensor(out=hq_f32[q][:], in0=hq_f32[q][:], in1=m2[:], op=ALU.add)
                    hb = hp.tile([H, 128], BF, tag="h_bf", name=f"h_bf_{q}_{l}")
                    nc.gpsimd.tensor_copy(out=hb[:], in_=hq_f32[q][:])
                    hq_bf[q] = hb

            # --- outputs ---
            h_all = cp.tile([H, NODES], F32, tag="h_all")
            for q in range(4):
                nc.gpsimd.tensor_copy(out=h_all[:, 128 * q:128 * (q + 1)], in_=hq_f32[q][:])
            pc = psA.tile([H, 1024], F32, tag="psA")
            mm(pc[0:3, 0:512], lhsT=c_coordw[:], rhs=h_all[:], start=True, stop=True)
            sb_c = wp.tile([3, NODES], F32, tag="sbc")
            nc.scalar.copy(sb_c[:], pc[0:3, 0:512])
            dma(out=out_coord, in_=sb_c[:])

            gf = wp.tile([H, G], F32, tag="gf")
            nc.vector.tensor_reduce(out=gf[:], in_=h_all[:].rearrange("p (g n) -> p g n", g=G),
                                    axis=X, op=ALU.add)
            pl_ = psA.tile([H, 1024], F32, tag="psA")
            mm(pl_[0:9, 0:G], lhsT=c_latws[:], rhs=gf[:], start=True, stop=True)
            sb_l = wp.tile([9, G], F32, tag="sbl")
            nc.vector.tensor_copy(out=sb_l[:], in_=pl_[0:9, 0:G])
            dma(out=out_lat, in_=sb_l[:])

    nc.finalize()
    return nc


def _host_prep(inputs):
    """Validate graph structure and build the per-core input maps."""
    t = np.asarray(inputs["t"], np.float32)
    frac = np.asarray(inputs["frac_coords"], np.float32)
    lattices = np.asarray(inputs["lattices"], np.float32)
    emb_table = np.asarray(inputs["emb_table"], np.float32)
    latent_w = np.asarray(inputs["latent_w"], np.float32)
    latent_b = np.asarray(inputs["latent_b"], np.float32)
    edge_w1 = np.asarray(inputs["edge_w1"], np.float32)
    edge_b1 = np.asarray(inputs["edge_b1"], np.float32)
    edge_w2 = np.asarray(inputs["edge_w2"], np.float32)
    edge_b2 = np.asarray(inputs["edge_b2"], np.float32)
    node_w1 = np.asarray(inputs["node_w1"], np.float32)
    node_b1 = np.asarray(inputs["node_b1"], np.float32)
    node_w2 = np.asarray(inputs["node_w2"], np.float32)
    node_b2 = np.asarray(inputs["node_b2"], np.float32)
    coord_w = np.asarray(inputs["coord_w"], np.float32)
    lattice_w = np.asarray(inputs["lattice_w"], np.float32)
    atom_types = np.asarray(inputs["atom_types"], np.int32)
    node2graph = np.asarray(inputs["node2graph"], np.int32)
    edge_index = np.asarray(inputs["edge_index"], np.int32)
    edge2graph = np.asarray(inputs["edge2graph"], np.int32)

    # structured-input check (dense per-graph edges, row-major)
    base = np.arange(N)
    ii = np.repeat(base, N)
    jj = np.tile(base, N)
    off = (np.arange(B) * N)[:, None]
    src_exp = (ii[None, :] + off).reshape(-1)
    dst_exp = (jj[None, :] + off).reshape(-1)
    structured = (
        np.array_equal(edge_index[0], src_exp)
        and np.array_equal(edge_index[1], dst_exp)
        and np.array_equal(node2graph, np.repeat(np.arange(B), N))
        and np.array_equal(edge2graph, np.repeat(np.arange(B), N * N))
    )
    if not structured:
        return None

    # gathers (host): embeddings + latent conditioning rows
    emb_g = emb_table[atom_types - 1]                      # [4096, H]
    t_rep = t[node2graph]                                  # [4096, LATENT]
    lat_ips = np.einsum("bij,bkj->bik", lattices, lattices).reshape(B, 9)

    # demb weight row permutation: device row r = 30s + 3f + c  <-  ref 30s+10c+f
    perm = np.empty(60, np.int64)
    for s in range(2):
        for f in range(NFREQ):
            for c in range(3):
                perm[30 * s + 3 * f + c] = 30 * s + 10 * c + f

    def dup64(a60):   # [60, H] -> [128, H] duplicated at partition 64
        out = np.zeros((H, H), np.float32)
        out[0:60] = a60
        out[64:124] = a60
        return out

    w1d_np = np.stack([dup64(edge_w1[l][265:325][perm]) for l in range(NLAYERS)])
    w1a_np = np.stack([edge_w1[l][0:H] for l in range(NLAYERS)])
    w1b_np = np.stack([edge_w1[l][H:2 * H] for l in range(NLAYERS)])
    w1c = np.stack([edge_w1[l][2 * H:2 * H + 9] for l in range(NLAYERS)])  # [L, 9, H]
    wn1h_np = np.stack([node_w1[l][0:H] for l in range(NLAYERS)])
    wn1a_np = np.stack([node_w1[l][H:2 * H] / np.float32(N) for l in range(NLAYERS)])

    freqs = np.arange(NFREQ, dtype=np.float64)             # integer freqs (2*pi folded out)

    in_maps = []
    for core in range(CORES):
        g0 = core * G
        nsl = slice(g0 * N, (g0 + G) * N)
        frac_c = frac[nsl]                                 # [512, 3]
        # sfs/sfd [128, GP*N]: col = 32*(pair) + node; partition<64 -> even graph
        sfs_np = np.zeros((H, GP * N), np.float32)
        sfd_np = np.zeros((H, GP * N), np.float32)
        for par in range(2):
            rbase = 64 * par
            for s in range(2):
                for f in range(NFREQ):
                    for c in range(3):
                        r = rbase + 30 * s + 3 * f + c
                        for p in range(GP):
                            nn0 = (2 * p + par) * N
                            x = frac_c[nn0:nn0 + N, c].astype(np.float64)
                            sfs_np[r, p * N:(p + 1) * N] = (freqs[f] * x).astype(np.float32)
                            sfd_np[r, p * N:(p + 1) * N] = (freqs[f] * x + (0.25 if s else 0.0)).astype(np.float32)

        cgb1_np = np.zeros((H, NLAYERS * G), np.float32)
        for l in range(NLAYERS):
            cg = lat_ips[g0:g0 + G] @ w1c[l] + edge_b1[l]  # [G, H]
            cgb1_np[:, G * l:G * (l + 1)] = cg.T

        m = {
            "embT": np.ascontiguousarray(emb_g[nsl].T).astype(BF16),
            "trep0": np.ascontiguousarray(t_rep[nsl, 0:H].T).astype(BF16),
            "trep1": np.ascontiguousarray(t_rep[nsl, H:LATENT].T).astype(BF16),
            "lw_e": latent_w[0:H].astype(BF16),
            "lw_t0": latent_w[H:2 * H].astype(BF16),
            "lw_t1": latent_w[2 * H:3 * H].astype(BF16),
            "latb": latent_b.reshape(H, 1),
            "sfs": sfs_np,
            "sfd": sfd_np,
            "w1d": w1d_np.astype(BF16),
            "w1a": w1a_np.astype(BF16),
            "w1b": w1b_np.astype(BF16),
            "w2": edge_w2.astype(BF16),
            "wn1h": wn1h_np.astype(BF16),
            "wn1a": wn1a_np.astype(BF16),
            "wn2": node_w2.astype(BF16),
            "cgb1": cgb1_np,
            "b2c": np.ascontiguousarray(edge_b2.T),
            "bn1c": np.ascontiguousarray(node_b1.T),
            "bn2c": np.ascontiguousarray(node_b2.T),
            "coordw": coord_w,
            "latws": lattice_w / np.float32(N),
        }
        in_maps.append(m)
    return in_maps, lattices


def _reference_fallback(inputs):
    """Pure-numpy fallback for non-structured inputs (not expected in practice)."""
    t = np.asarray(inputs["t"], np.float32)
    frac = np.asarray(inputs["frac_coords"], np.float32)
    lattices = np.asarray(inputs["lattices"], np.float32)
    silu = lambda x: x / (1.0 + np.exp(-x))
    emb = np.asarray(inputs["emb_table"], np.float32)
    atom_types = np.asarray(inputs["atom_types"], np.int32)
    node2graph = np.asarray(inputs["node2graph"], np.int32)
    edge_index = np.asarray(inputs["edge_index"], np.int32)
    edge2graph = np.asarray(inputs["edge2graph"], np.int32)
    src, dst = edge_index
    nnodes = frac.shape[0]
    h = emb[atom_types - 1]
    h = np.concatenate([h, t[node2graph]], 1) @ np.asarray(inputs["latent_w"]) + np.asarray(inputs["latent_b"])
    fd = np.mod(frac[dst] - frac[src], 1.0)
    freqs = 2 * np.pi * np.arange(NFREQ, dtype=np.float32)
    demb = (fd[..., None] * freqs).reshape(-1, NFREQ * 3)
    demb = np.concatenate([np.sin(demb), np.cos(demb)], -1)
    lat_ips = np.einsum("bij,bkj->bik", lattices, lattices).reshape(-1, 9)
    lat_e = lat_ips[edge2graph]
    deg = np.maximum(np.bincount(src, minlength=nnodes).astype(np.float32), 1.0)
    for l in range(NLAYERS):
        e = np.concatenate([h[src], h[dst], lat_e, demb], 1)
        e = silu(e @ np.asarray(inputs["edge_w1"])[l] + np.asarray(inputs["edge_b1"])[l])
        e = silu(e @ np.asarray(inputs["edge_w2"])[l] + np.asarray(inputs["edge_b2"])[l])
        agg = np.zeros((nnodes, H), np.float32)
        np.add.at(agg, src, e)
        agg = agg / deg[:, None]
        m = np.concatenate([h, agg], 1)
        m = silu(m @ np.asarray(inputs["node_w1"])[l] + np.asarray(inputs["node_b1"])[l])
        m = silu(m @ np.asarray(inputs["node_w2"])[l] + np.asarray(inputs["node_b2"])[l])
        h = h + m
    coord_out = h @ np.asarray(inputs["coord_w"])
    gcount = np.maximum(np.bincount(node2graph, minlength=B).astype(np.float32), 1.0)
    gfeat = np.zeros((B, H), np.float32)
    np.add.at(gfeat, node2graph, h)
    gfeat = gfeat / gcount[:, None]
    lattice_out = (gfeat @ np.asarray(inputs["lattice_w"])).reshape(-1, 3, 3)
    lattice_out = np.einsum("bij,bjk->bik", lattice_out, lattices)
    return lattice_out.astype(np.float32), coord_out.astype(np.float32)


def kernel(**inputs):
    global _BUILT, LAST_RESULTS
    prep = _host_prep(inputs)
    if prep is None:
        return _reference_fallback(inputs)
    in_maps, lattices = prep

    from concourse.bass_utils import run_bass_kernel_spmd

    if _BUILT is None:
        _BUILT = _build_nc()
    nc = _BUILT

    res = run_bass_kernel_spmd(nc, in_maps, core_ids=list(range(CORES)), trace=TRACE)
    LAST_RESULTS = res

    coord_out = np.empty((B * N, 3), np.float32)
    lattice_out = np.empty((B, 3, 3), np.float32)
    for core in range(CORES):
        r = res.results[core]
        coord_out[core * NODES:(core + 1) * NODES] = r["out_coord"].T
        pre = np.ascontiguousarray(r["out_lat"].T).reshape(G, 3, 3)
        gsl = slice(core * G, (core + 1) * G)
        lattice_out[gsl] = np.einsum("bij,bjk->bik", pre, lattices[gsl])
    return lattice_out, coord_out
